# revision 1
# baseline (speedup 1.0000x reference)
"""MoE feed-forward (top-1 routing) on 8 TRN2 NeuronCores.

Sharding: tensor-parallel over D_FF on top of the expert dim. Core c holds
f-columns [c*512:(c+1)*512] of EVERY expert's w1/b1/w2 and processes the
full expert-sorted token stream, emitting a partial y; the host sums the 8
partials and adds b2. This makes the per-core work identical regardless of
how the router balances tokens (no expert-parallel load imbalance).

Host does the gate (tiny matmul) + dispatch/combine (the "all-to-all").
Device: y_part^T = w2s^T @ relu(w1s^T @ x^T + b1s), tokens kept in the
matmul free dimension throughout, so no on-device transposes. All weights
stay resident in SBUF as bf16.
"""

import os

import numpy as np
import ml_dtypes

import concourse.bass as bass
from concourse import bacc
import concourse.mybir as mybir
from concourse.tile import TileContext
from concourse.bass_utils import run_bass_kernel_spmd

P = 128
D_MODEL = 1024
D_FF = 4096
NUM_EXPERTS = 8
KD = D_MODEL // P   # 8  d-tiles
FH = D_FF // 8      # 512 f-columns per core
KH = FH // P        # 4  f-tiles per expert-slice

BF16 = mybir.dt.bfloat16
F32 = mybir.dt.float32


def _seg_chunks(C, first_small):
    """Split C into chunk widths <=512, avoiding tiny tails (<128)."""
    sizes = []
    rem = C
    if first_small and rem > 256:
        # small first chunk so the PE can start as soon as ~0.5MB has landed
        sizes.append(256)
        rem -= 256
    while rem > 576:
        sizes.append(512)
        rem -= 512
    if rem > 512:
        a = -(-(rem // 2) // 16) * 16
        sizes += [a, rem - a]
    elif rem:
        sizes.append(rem)
    return sizes


def _build(caps):
    nc = bacc.Bacc()
    CT = sum(caps)
    xT = nc.declare_dram_parameter("xT", [P, KD, CT], BF16, isOutput=False)
    w1 = nc.declare_dram_parameter("w1", [P, KD, D_FF], BF16, isOutput=False)
    b1 = nc.declare_dram_parameter("b1", [P, 8 * KH], F32, isOutput=False)
    w2 = nc.declare_dram_parameter("w2", [P, 8 * KH, D_MODEL], BF16, isOutput=False)
    out = nc.declare_dram_parameter("out", [P, KD, CT], BF16, isOutput=True)

    with TileContext(nc) as tc:
        with (
            tc.tile_pool(name="wpool", bufs=1) as wpool,
            tc.tile_pool(name="xpool", bufs=3) as xpool,
            tc.tile_pool(name="hpool", bufs=2) as hpool,
            tc.tile_pool(name="ypool", bufs=2) as ypool,
            tc.tile_pool(name="ps1", bufs=4, space="PSUM") as ps1pool,
            tc.tile_pool(name="ps2", bufs=4, space="PSUM") as ps2pool,
        ):
            # global chunk list: (expert_seg, global_col0, width)
            work = []
            off = 0
            for s in range(8):
                if caps[s] == 0:
                    continue
                sizes = _seg_chunks(caps[s], first_small=(len(work) == 0))
                c0 = 0
                for wdt in sizes:
                    work.append((s, off + c0, wdt))
                    c0 += wdt
                off += caps[s]

            # x for the first two chunks before any weight DMA; spread DMA
            # triggers across engine queues (issue is ~1us each, serialized
            # per queue).
            x_tiles = {}
            for wi, (s, g0, cw) in enumerate(work[:2]):
                x_sb = xpool.tile([P, KD, 512], BF16, tag="x")
                # scalar = HWDGE queue, keeps sync free for the first w1 slice
                nc.scalar.dma_start(x_sb[:, :, :cw], xT[:, :, g0:g0 + cw])
                x_tiles[wi] = x_sb

            b1_sb = wpool.tile([P, 8 * KH], F32, tag="b1")
            nc.scalar.dma_start(b1_sb[:], b1[:])


            # Resident weights: per expert-segment slices, interleaved in
            # the order compute consumes them (w1_s before w2_s). Each 1MB
            # slice is split across two DMA queues (a single queue moves
            # ~45GB/s); triggers go on gpsimd, which is otherwise idle.
            w1_t, w2_t = [], []
            for s in range(8):
                t1 = wpool.tile([P, KD, FH], BF16, tag=f"w1_{s}")
                if s == 0:
                    # first slice on sync (HWDGE): lower first-byte latency
                    # than gpsimd's software DGE, and this DMA gates the
                    # very first matmul
                    nc.sync.dma_start(t1[:, :, :128], w1[:, :, :128])
                    nc.sync.dma_start(t1[:, :, 128:256], w1[:, :, 128:256])
                    nc.gpsimd.dma_start(t1[:, :, 256:], w1[:, :, 256:FH])
                else:
                    h = FH // 2
                    o = s * FH
                    nc.gpsimd.dma_start(t1[:, :, :h], w1[:, :, o:o + h])
                    nc.gpsimd.dma_start(t1[:, :, h:], w1[:, :, o + h:o + FH])
                w1_t.append(t1)
                t2 = wpool.tile([P, KH, D_MODEL], BF16, tag=f"w2_{s}")
                nc.gpsimd.dma_start(t2[:, :2], w2[:, s * KH:s * KH + 2])
                nc.gpsimd.dma_start(t2[:, 2:], w2[:, s * KH + 2:(s + 1) * KH])
                w2_t.append(t2)

            for wi, (s, g0, cw) in enumerate(work):
                if wi in x_tiles:
                    x_sb = x_tiles[wi]
                else:
                    x_sb = xpool.tile([P, KD, 512], BF16, tag="x")
                    nc.sync.dma_start(x_sb[:, :, :cw], xT[:, :, g0:g0 + cw])

                h_sb = hpool.tile([P, KH, 512], BF16, tag="h")
                # FFN1: H^T[fo] = relu(w1s[:, fo]^T @ x^T + b1s[fo])
                for fo in range(KH):
                    ps = ps1pool.tile([P, 512], F32, tag="ps1")
                    for ko in range(KD):
                        nc.tensor.matmul(
                            ps[:, :cw],
                            w1_t[s][:, ko, fo * P:(fo + 1) * P],
                            x_sb[:, ko, :cw],
                            start=(ko == 0),
                            stop=(ko == KD - 1),
                        )
                    nc.scalar.activation(
                        h_sb[:, fo, :cw],
                        ps[:, :cw],
                        mybir.ActivationFunctionType.Relu,
                        bias=b1_sb[:, s * KH + fo:s * KH + fo + 1],
                    )
                # FFN2 partial: y^T[do] = w2s[:, do]^T @ H^T  (b2 on host)
                last = wi == len(work) - 1
                y_sb = ypool.tile([P, KD, 512], BF16, tag="y")
                for do in range(KD):
                    ps2 = ps2pool.tile([P, 512], F32, tag="ps2")
                    for fo in range(KH):
                        nc.tensor.matmul(
                            ps2[:, :cw],
                            w2_t[s][:, fo, do * P:(do + 1) * P],
                            h_sb[:, fo, :cw],
                            start=(fo == 0),
                            stop=(fo == KH - 1),
                        )
                    nc.vector.tensor_copy(y_sb[:, do, :cw], ps2[:, :cw])
                    if last:
                        # stream the tail out per do-group to shorten the drain
                        nc.sync.dma_start(out[:, do, g0:g0 + cw], y_sb[:, do, :cw])
                if not last:
                    nc.sync.dma_start(out[:, :, g0:g0 + cw], y_sb[:, :, :cw])
    nc.compile()
    return nc


_NC_CACHE = {}
LAST_EXEC_NS = None


def _get_nc(caps):
    if caps not in _NC_CACHE:
        _NC_CACHE[caps] = _build(caps)
    return _NC_CACHE[caps]


def _part3(a, kd):
    # [kd*P, cols...] -> [P, kd, cols] partition-inner layout
    return np.ascontiguousarray(
        a.reshape(kd, P, a.shape[1]).transpose(1, 0, 2))


def kernel(x, gate_w, gate_b, expert_bias, w1, b1, w2, b2):
    global LAST_EXEC_NS
    B, S, D = x.shape
    xf = np.ascontiguousarray(x.reshape(-1, D)).astype(np.float32)

    logits = xf @ gate_w.T.astype(np.float32) + (gate_b + expert_bias)
    top = logits.argmax(-1)

    counts = np.bincount(top, minlength=NUM_EXPERTS)
    caps = tuple(int(-(-c // 16) * 16) for c in counts)
    CT = sum(caps)

    # Expert-sorted padded token stream, shared by all cores.
    idx_lists = []
    xg = np.zeros((CT, D), np.float32)
    off = 0
    offs = []
    for e in range(NUM_EXPERTS):
        ids = np.nonzero(top == e)[0]
        idx_lists.append(ids)
        offs.append(off)
        xg[off:off + len(ids)] = xf[ids]
        off += caps[e]
    xT = _part3(np.ascontiguousarray(xg.T).astype(ml_dtypes.bfloat16), KD)

    w1f = np.asarray(w1, np.float32)
    w2f = np.asarray(w2, np.float32)
    b1f = np.asarray(b1, np.float32)

    in_maps = []
    for c in range(NUM_EXPERTS):
        fs = slice(c * FH, (c + 1) * FH)
        # pack every expert's f-slice side by side
        w1c = np.concatenate([w1f[e][:, fs] for e in range(NUM_EXPERTS)],
                             axis=1).astype(ml_dtypes.bfloat16)   # [D, 8*FH]
        w2c = np.concatenate([w2f[e][fs, :] for e in range(NUM_EXPERTS)],
                             axis=0).astype(ml_dtypes.bfloat16)   # [8*FH, D]
        b1c = np.stack([b1f[e][fs] for e in range(NUM_EXPERTS)])  # [8, FH]
        in_maps.append({
            "xT": xT,
            "w1": _part3(w1c, KD),
            "w2": _part3(w2c, 8 * KH),
            "b1": np.ascontiguousarray(b1c.reshape(8 * KH, P).T),
        })

    nc = _get_nc(caps)
    res = None
    for attempt in range(3):
        try:
            res = run_bass_kernel_spmd(nc, in_maps, list(range(NUM_EXPERTS)))
            break
        except Exception:
            # rare transient NRT_EXEC_UNIT_UNRECOVERABLE from the runtime;
            # a straight retry has been observed to succeed
            if attempt == 2:
                raise
            import time
            time.sleep(5)
    LAST_EXEC_NS = res.exec_time_ns

    acc = np.zeros((P, KD, CT), np.float32)
    for c in range(NUM_EXPERTS):
        acc += np.asarray(res.results[c]["out"]).astype(np.float32)
    yg = acc.transpose(1, 0, 2).reshape(D, CT).T   # [CT, D]

    out = np.zeros_like(xf)
    for e in range(NUM_EXPERTS):
        ids = idx_lists[e]
        if len(ids):
            out[ids] = yg[offs[e]:offs[e] + len(ids)] + b2[e]
    return out.reshape(B, S, D)



# revision 32
# speedup vs baseline: 1.3206x; 1.3206x over previous
"""MoE feed-forward (top-1 routing) on 8 TRN2 NeuronCores.

Sharding: tensor-parallel over D_FF. Core c holds f-columns
[c*512:(c+1)*512] of EVERY expert's w1/b1/w2 and processes the full
expert-sorted token stream, emitting a partial y; the host sums the 8
partials and adds b2. Per-core work is identical regardless of routing.

Matmuls run as fp8(e4m3) DoubleRow with a hi/lo residue decomposition:
every operand v is shipped as vh = e4m3(s*v), vl = e4m3(s*v - vh), and
each logical product keeps 3 of the 4 cross terms (vh*wh + vh*wl +
vl*wh); the dropped lo*lo term is ~2^-9 relative. DoubleRow packs two
128-row contraction slices per instruction, so the 3-term scheme still
beats bf16 by 4/3 while matching bf16 accuracy.

Scales: x*16, w1*1024, w2*1024; h is produced as 16*h by the relu
(scale 2^-10 on the 16384x psum), so the FFN2 psum is 16384x the true
partial and the host divides once during the combine.

Schedule notes (all tuned against the CoreSim cost model, which is
what the reported exec time tracks):
- PE pipeline: FFN2 of chunk i issues after FFN1 of chunk i+1, so the
  scalar/vector h-decomposition chain never stalls the tensor engine.
- Tiny warm-up matmuls at t=0 (and small bridges at known DMA wait
  points) keep the PE p-state ramped while data lands.
- Chunk 0 and expert 0's w1 are packed kp-major and loaded in quarter
  blocks, interleaved SP/Activation, so the first matmul starts as
  soon as one contraction quarter is resident.
- DMA issue occupies the issuing engine ~0.8-1.6us and all transfers
  serialize on one 360GB/s resource, so transfers are few and dense:
  per-chunk x/out tensors padded to 512 columns move as single
  full-tile DMAs.  Pool-engine weight loads are chained behind a tiny
  corner-copy gated on chunk 0's output so they cannot preempt the
  head-critical stream; x prefetches are throttled by their ring.
"""

import numpy as np
import ml_dtypes

from concourse import bacc
import concourse.mybir as mybir
from concourse.tile import TileContext
from concourse.bass_utils import run_bass_kernel_spmd

P = 128
D_MODEL = 1024
D_FF = 4096
NUM_EXPERTS = 8
KD = D_MODEL // P   # 8  d-tiles
FH = D_FF // 8      # 512 f-columns per core
KH = FH // P        # 4  f-tiles per expert-slice

E4 = mybir.dt.float8e4
BF16 = mybir.dt.bfloat16
F16 = mybir.dt.float16
F32 = mybir.dt.float32
DR = mybir.MatmulPerfMode.DoubleRow
RELU = mybir.ActivationFunctionType.Relu
COPY = mybir.ActivationFunctionType.Copy
SUB = mybir.AluOpType.subtract

SX = 16.0       # x scale
SW = 1024.0     # w1/w2 scale
SH = 16.0       # h scale (relu output = SH * h)
PS1_SCALE = SH / (SX * SW)          # 2^-10: psum1 -> 16*h
OUT_SCALE = SH * SW                 # 16384: psum2 = OUT_SCALE * y_partial

N_WARM = 65          # head warm-up matmuls (tuned against the sim)
MID_WARM = (6, 7, 6)  # bridges after chunk0's kp0/kp1/kp2 blocks
B_X1 = 6             # bridge before FFN1(1)
B_W2 = 24            # bridge before FFN2(0)
W_FREE = 64          # out-free of each warm-up matmul

# kp-major row permutation: [hi0 hi1 lo0 lo1 | hi2 hi3 lo2 lo3 | ...]
KP_IDX = [0, 1, 8, 9, 2, 3, 10, 11, 4, 5, 12, 13, 6, 7, 14, 15]


def _seg_chunks(C, first):
    """Split C columns into chunk widths <=512."""
    sizes = []
    rem = C
    while rem > 576:
        sizes.append(512)
        rem -= 512
    if rem > 512:
        a = -(-(rem // 2) // 16) * 16
        sizes += [a, rem - a]
    elif rem:
        sizes.append(rem)
    return sizes


def _work_list(caps):
    work = []
    off = 0
    for s in range(8):
        if caps[s] == 0:
            continue
        for w in _seg_chunks(caps[s], first=(len(work) == 0)):
            work.append((s, off, w))
            off += w
    # small final chunks shorten the drain after the last matmul
    if work[-1][2] > 352:
        s, g0, cw = work.pop()
        work.append((s, g0, cw - 192))
        work.append((s, g0 + cw - 192, 128))
        work.append((s, g0 + cw - 64, 64))
    elif work[-1][2] > 96:
        s, g0, cw = work.pop()
        work.append((s, g0, cw - 64))
        work.append((s, g0 + cw - 64, 64))
    return work


def _build(caps):
    nc = bacc.Bacc()
    work = _work_list(caps)
    NW = len(work)
    s0 = work[0][0]
    cw_last = work[-1][2]

    def xdr_shape(wi):
        cw = work[wi][2]
        if wi == 0:
            return [P, 16, cw]          # exact (head-critical)
        return [P, 16, 256 if cw <= 256 else 512]   # padded, host zero-fills

    x_dr = [nc.declare_dram_parameter(f"x{wi}", xdr_shape(wi), E4,
                                      isOutput=False) for wi in range(NW)]
    o_dr = [nc.declare_dram_parameter(f"o{wi}", [P, KD, work[wi][2]], BF16,
                                      isOutput=True) for wi in range(NW)]
    w1h0 = nc.declare_dram_parameter("w1h0", [P, 16, FH], E4, isOutput=False)
    w1 = nc.declare_dram_parameter("w1", [P, 2 * KD, D_FF], E4, isOutput=False)
    b1 = nc.declare_dram_parameter("b1", [P, 8 * KH], F32, isOutput=False)
    w2 = nc.declare_dram_parameter("w2", [P, 8 * 2 * KH, D_MODEL], E4,
                                   isOutput=False)

    # first chunk index of each segment, for weight prefetch pacing
    seg_first = {}
    for wi, (s, g0, cw) in enumerate(work):
        seg_first.setdefault(s, wi)
    segs = sorted(seg_first, key=seg_first.get)

    def xrow(wi, kp, lo):
        if wi == 0:
            return 4 * kp + (2 if lo else 0)
        return (8 if lo else 0) + 2 * kp

    def wrow(s, kp, lo):
        if s == s0:
            return 4 * kp + (2 if lo else 0)
        return (8 if lo else 0) + 2 * kp

    with TileContext(nc) as tc:
        with (
            tc.tile_pool(name="wpool", bufs=1) as wpool,
            tc.tile_pool(name="xpool", bufs=3) as xpool,
            tc.tile_pool(name="hpool", bufs=2) as hpool,
            tc.tile_pool(name="ypool", bufs=2) as ypool,
            tc.tile_pool(name="psp", bufs=8, space="PSUM") as pspool,
        ):
            # --- PE warm-up: ramp the p-state while the first DMAs land ---
            wzx = wpool.tile([P, 2, P + W_FREE], E4, tag="wzx")
            nc.vector.memset(wzx[:], 0.0)
            psw = pspool.tile([P, W_FREE], F32, tag="ps")

            def warm(n):
                for _ in range(n):
                    nc.tensor.matmul(psw[:], wzx[:, :, :P],
                                     wzx[:, :, P:], start=True,
                                     stop=True, perf_mode=DR)

            warm(N_WARM)

            # --- head DMAs (HWDGE): w1h0 quarters on SP, x0 quarters on
            # Activation, then x1/w2s0/x2/b1 ---
            cw0 = work[0][2]
            t1_0 = wpool.tile([P, 16, FH], E4, tag=f"w1_{s0}")
            x_sb0 = xpool.tile([P, 16, 512], E4, tag="x")
            for kp in range(4):
                nc.sync.dma_start(t1_0[:, 4 * kp:4 * kp + 4, :],
                                  w1h0[:, 4 * kp:4 * kp + 4, :])
                nc.scalar.dma_start(x_sb0[:, 4 * kp:4 * kp + 4, :cw0],
                                    x_dr[0][:, 4 * kp:4 * kp + 4, :])
            x_tiles = {0: x_sb0}
            if NW > 1:
                x_sb1 = xpool.tile([P, 16, 512], E4, tag="x")
                nc.sync.dma_start(x_sb1[:], x_dr[1][:])
                x_tiles[1] = x_sb1
            w1_t, w2_t = {s0: t1_0}, {}
            t2_0 = wpool.tile([P, 2 * KH, D_MODEL], E4, tag=f"w2_{s0}")
            nc.scalar.dma_start(t2_0[:], w2[:, 8 * s0:8 * s0 + 8])
            w2_t[s0] = t2_0
            if NW > 2:
                x_sb2 = xpool.tile([P, 16, 512], E4, tag="x")
                nc.sync.dma_start(x_sb2[:], x_dr[2][:])
                x_tiles[2] = x_sb2
            b1_sb = wpool.tile([P, 8 * KH], F32, tag="b1")
            nc.scalar.dma_start(b1_sb[:], b1[:])

            def ffn1(wi):
                s, g0, cw = work[wi]
                x_sb = x_tiles[wi]
                h32 = hpool.tile([P, KH, 512], F16, tag="h32")
                hh = hpool.tile([P, KH, 512], E4, tag="hh")
                hl = hpool.tile([P, KH, 512], E4, tag="hl")
                t1 = w1_t[s]
                ps = [pspool.tile([P, 512], F32, tag="ps", name=f"ps1_{fo}")
                      for fo in range(KH)]

                def mm(fo, kp, term, start, stop):
                    wlo, xlo = ((False, False), (True, False),
                                (False, True))[term]
                    wr = wrow(s, kp, wlo)
                    xr = xrow(wi, kp, xlo)
                    fsl = slice(fo * P, (fo + 1) * P)
                    nc.tensor.matmul(
                        ps[fo][:, :cw],
                        t1[:, wr:wr + 2, fsl],
                        x_sb[:, xr:xr + 2, :cw],
                        start=start, stop=stop, perf_mode=DR,
                    )

                if wi == 0:
                    # K-outer so compute starts on the first kp block
                    for kp in range(4):
                        for term in range(3):
                            for fo in range(KH):
                                mm(fo, kp, term,
                                   start=(kp == 0 and term == 0),
                                   stop=(kp == 3 and term == 2))
                        if kp < 3:
                            warm(MID_WARM[kp])
                else:
                    for fo in range(KH):
                        k = 0
                        for term in range(3):
                            for kp in range(4):
                                mm(fo, kp, term, start=(k == 0), stop=(k == 11))
                                k += 1
                # h decomposition: hh first (FFN2 T1/T2 gate on it)
                for fo in range(KH):
                    bsl = b1_sb[:, s * KH + fo:s * KH + fo + 1]
                    nc.scalar.activation(hh[:, fo, :cw], ps[fo][:, :cw], RELU,
                                         bias=bsl, scale=PS1_SCALE)
                    nc.scalar.activation(h32[:, fo, :cw], ps[fo][:, :cw], RELU,
                                         bias=bsl, scale=PS1_SCALE)
                    nc.vector.tensor_tensor(hl[:, fo, :cw], h32[:, fo, :cw],
                                            hh[:, fo, :cw], SUB)
                return hh, hl

            def ffn2(wi, hh, hl):
                s, g0, cw = work[wi]
                last = wi >= NW - 2 and cw <= 160 and wi >= 1
                t2 = w2_t[s]
                if last:
                    y_sb = ypool.tile([P, KD, cw], BF16, tag=f"ylast{cw}")
                else:
                    y_sb = ypool.tile([P, KD, 512], BF16, tag="y")
                for do in range(KD):
                    ps2 = pspool.tile([P, 512], F32, tag="ps")
                    dsl = slice(do * P, (do + 1) * P)
                    k = 0
                    for kp in range(0, KH, 2):
                        for wr, hx in ((0, hh), (KH, hh), (0, hl)):
                            nc.tensor.matmul(
                                ps2[:, :cw],
                                t2[:, wr + kp:wr + kp + 2, dsl],
                                hx[:, kp:kp + 2, :cw],
                                start=(k == 0), stop=(k == 5), perf_mode=DR,
                            )
                            k += 1
                    on_act = (do % 2 == 0) if last else (do < 1)
                    if on_act:
                        nc.scalar.activation(y_sb[:, do, :cw], ps2[:, :cw],
                                             COPY)
                    else:
                        nc.vector.tensor_copy(y_sb[:, do, :cw], ps2[:, :cw])
                    split = last or (wi >= NW - 6 and cw >= 256)
                    if split and do == KD // 2 - 1:
                        nc.sync.dma_start(o_dr[wi][:, :KD // 2],
                                          y_sb[:, :KD // 2, :cw])
                if split:
                    nc.sync.dma_start(o_dr[wi][:, KD // 2:],
                                      y_sb[:, KD // 2:, :cw])
                else:
                    nc.sync.dma_start(o_dr[wi][:], y_sb[:, :, :cw])

            # ---- chunk 0 ----
            hh0, hl0 = ffn1(0)

            # --- gated pool chain: expert weights for segments >= 1, each
            # link's corner pre-written so the chain cannot start before
            # chunk 0's FFN1 output exists and stays strictly ordered ---
            prev_gate = hh0
            for s in segs[1:]:
                t1 = wpool.tile([P, 16, FH], E4, tag=f"w1_{s}")
                nc.vector.tensor_copy(t1[:, 0:1, 0:2], prev_gate[:, 0:1, 0:2])
                o = s * FH
                nc.gpsimd.dma_start(t1[:], w1[:, :, o:o + FH])
                w1_t[s] = t1
                t2 = wpool.tile([P, 2 * KH, D_MODEL], E4, tag=f"w2_{s}")
                nc.vector.tensor_copy(t2[:, 0:1, 0:2], t1[:, 0:1, 0:2])
                nc.gpsimd.dma_start(t2[:], w2[:, 2 * KH * s:2 * KH * (s + 1)])
                w2_t[s] = t2
                prev_gate = t2

            # x prefetches for wi >= 3 on SP; the 3-deep ring throttles
            # them (each tile's buffer is WAW-bound to release of wi-3),
            # except x3 which is corner-gated on chunk 0's output.
            for wi in range(3, NW):
                wpad = xdr_shape(wi)[2]
                x_sb = xpool.tile([P, 16, wpad], E4,
                                  tag="xs" if wpad == 256 else "x",
                                  name=f"xt{wi}")
                if wi == 3:
                    nc.vector.tensor_copy(x_sb[:, 0:1, 0:2],
                                          hh0[:, 0:1, 0:2])
                nc.sync.dma_start(x_sb[:], x_dr[wi][:])
                x_tiles[wi] = x_sb

            # ---- pipeline ----
            warm(B_X1)
            pend = (0, hh0, hl0)
            for wi in range(1, NW):
                h = ffn1(wi)
                if wi == 1:
                    warm(B_W2)
                ffn2(pend[0], pend[1], pend[2])
                pend = (wi, h[0], h[1])
            ffn2(pend[0], pend[1], pend[2])
    nc.compile()
    return nc


_NC_CACHE = {}
_W_CACHE = {}
LAST_EXEC_NS = None


def _get_nc(caps):
    if caps not in _NC_CACHE:
        _NC_CACHE[caps] = _build(caps)
    return _NC_CACHE[caps]


def _part3(a, kd):
    # [kd*P, cols...] -> [P, kd, cols] partition-inner layout
    return np.ascontiguousarray(
        a.reshape(kd, P, a.shape[1]).transpose(1, 0, 2))


def _hilo(a):
    """[P, kd, cols] f32 -> [P, 2*kd, cols] e4m3 (hi rows then lo rows)."""
    hi = a.astype(ml_dtypes.float8_e4m3)
    lo = (a - hi.astype(np.float32)).astype(ml_dtypes.float8_e4m3)
    return np.concatenate([hi, lo], axis=1)


def _prep_weights(w1, b1, w2):
    key = (id(w1), id(b1), id(w2))
    if key in _W_CACHE:
        return _W_CACHE[key]
    w1f = np.asarray(w1, np.float32) * SW
    w2f = np.asarray(w2, np.float32) * SW
    b1f = np.asarray(b1, np.float32) * SH
    per_core = []
    for c in range(NUM_EXPERTS):
        fs = slice(c * FH, (c + 1) * FH)
        w1c = np.concatenate([w1f[e][:, fs] for e in range(NUM_EXPERTS)],
                             axis=1)                       # [D, 8*FH]
        w1q = _hilo(_part3(w1c, KD))                       # [P, 16, 4096]
        w2qs = []
        for e in range(NUM_EXPERTS):
            w2e = _part3(np.ascontiguousarray(w2f[e][fs, :]), KH)  # [P,KH,D]
            w2qs.append(_hilo(w2e))                        # [P, 8, D]
        w2q = np.concatenate(w2qs, axis=1)                 # [P, 64, D]
        b1c = np.stack([b1f[e][fs] for e in range(NUM_EXPERTS)])   # [8, FH]
        per_core.append({
            "w1": w1q,
            "w2": w2q,
            "b1": np.ascontiguousarray(b1c.reshape(8 * KH, P).T),
        })
    _W_CACHE.clear()
    _W_CACHE[key] = per_core
    return per_core


def kernel(x, gate_w, gate_b, expert_bias, w1, b1, w2, b2):
    global LAST_EXEC_NS
    B, S, D = x.shape
    xf = np.ascontiguousarray(x.reshape(-1, D)).astype(np.float32)

    logits = xf @ gate_w.T.astype(np.float32) + (gate_b + expert_bias)
    top = logits.argmax(-1)

    counts = np.bincount(top, minlength=NUM_EXPERTS)
    caps = tuple(int(-(-c // 2) * 2) for c in counts)
    CT = sum(caps)

    # Expert-sorted padded token stream, shared by all cores.
    idx_lists = []
    xg = np.zeros((CT, D), np.float32)
    off = 0
    offs = []
    for e in range(NUM_EXPERTS):
        ids = np.nonzero(top == e)[0]
        idx_lists.append(ids)
        offs.append(off)
        xg[off:off + len(ids)] = xf[ids]
        off += caps[e]
    xTq = _hilo(_part3(np.ascontiguousarray(xg.T) * SX, KD))  # [P,16,CT]

    work = _work_list(caps)
    NW = len(work)
    x_chunks = {}
    for wi, (s, g0, cw) in enumerate(work):
        blk = xTq[:, :, g0:g0 + cw]
        if wi == 0:
            blk = blk[:, KP_IDX, :]
        wpad = 256 if cw <= 256 else 512
        if wi >= 1 and cw < wpad:
            pad = np.zeros((P, 16, wpad), xTq.dtype)
            pad[:, :, :cw] = blk
            blk = pad
        x_chunks[f"x{wi}"] = np.ascontiguousarray(blk)

    per_core_w = _prep_weights(w1, b1, w2)
    s0 = work[0][0]
    in_maps = []
    for c in range(NUM_EXPERTS):
        m = dict(per_core_w[c])
        m["w1h0"] = np.ascontiguousarray(
            m["w1"][:, :, s0 * FH:(s0 + 1) * FH][:, KP_IDX, :])
        m.update(x_chunks)
        in_maps.append(m)

    nc = _get_nc(caps)
    res = None
    for attempt in range(3):
        try:
            res = run_bass_kernel_spmd(nc, in_maps, list(range(NUM_EXPERTS)))
            break
        except Exception:
            # rare transient NRT_EXEC_UNIT_UNRECOVERABLE from the runtime;
            # a straight retry has been observed to succeed
            if attempt == 2:
                raise
            import time
            time.sleep(5)
    LAST_EXEC_NS = res.exec_time_ns

    acc = np.zeros((P, KD, CT), np.float32)
    for c in range(NUM_EXPERTS):
        for wi, (s, g0, cw) in enumerate(work):
            acc[:, :, g0:g0 + cw] += np.asarray(
                res.results[c][f"o{wi}"]).astype(np.float32)[:, :, :cw]
    acc *= 1.0 / OUT_SCALE
    yg = acc.transpose(1, 0, 2).reshape(D, CT).T   # [CT, D]

    out = np.zeros_like(xf)
    for e in range(NUM_EXPERTS):
        ids = idx_lists[e]
        if len(ids):
            out[ids] = yg[offs[e]:offs[e] + len(ids)] + b2[e]
    return out.reshape(B, S, D)
